# revision 36
# baseline (speedup 1.0000x reference)
"""Trainium2 Bass kernel for nn_ExpertsLinear (weighted mixture of 8 experts).

    y[b, o] = sum_e weights[b, e] * (x @ W[e] + b[e])[b, o]

Split-precision formulation. The gate matrix w [B, 8] is split host-side
via rank-2 SVD: w = G @ V + res (G = U[:, :2]*S[:2], V = Vt[:2]).

    y_b = sum_r G_br * (x_b @ W'_r)            # fp16, W'_r = sum_e V_re W_e
        + sum_e res_be * (x_b @ W_e)           # fp8-e4m3 DoubleRow, 2x rate

The fp16 term carries ~87% of the signal; the fp8 residual term's
quantization error lands at l2_rel ~1.7e-2 (gate 2e-2, simulated with
exact kernel quantization). DoubleRow packs 2 fp8 k-values per PE cell:
lhsT [K,2,M], rhs [K,2,N], contraction over (k, pair) — verified on HW.

Host-side preprocessing: SVD of w; x pre-transposed fp16; residual-gated
x pre-scaled (*32, clip +-240) and packed fp8; W packed fp8 (*2^15);
pseudo-expert weights/gates fp16. Scales divided out at PSUM evacuation
(ACT scale-copy + DVE add), y stored fp16.

Per-core, per 128-row tile: 8 fp16 MMs (2 pseudo-experts) into one PSUM
bank + 16 DoubleRow fp8 MMs (8 residual experts) into a second bank,
then y = ps_main + ps_delta * 2^-20. Head: expert-outer rounds over
HOIST tiles while weights stream; zero-matmul prewarm bridges the
initial all-cores HBM burst and warms the HAM clock gate.
"""

import numpy as np

P = 128
D = 512
E = 8
R = 2
FC = D // P
N_CORES = 8
B_FULL = 65536
B_LOC = B_FULL // N_CORES
NBT = B_LOC // P

HOIST = 4
NWARM = 11
SX = 32.0
SW = 2.0 ** 15
EVAC = 1.0 / (SX * SW)

_COMPILED = {}


def _build_nc():
    import concourse.bacc as bacc
    import concourse.mybir as mybir
    import concourse.tile as tile

    F32 = mybir.dt.float32
    F16 = mybir.dt.float16
    F8 = mybir.dt.float8e4
    DR = mybir.MatmulPerfMode.DoubleRow

    nc = bacc.Bacc(
        "TRN2",
        target_bir_lowering=False,
        debug=False,
        enable_asserts=False,
        num_devices=N_CORES,
    )
    xt_d = nc.dram_tensor("XT", [P, NBT, FC, P], F16, kind="ExternalInput").ap()
    g_d = nc.dram_tensor("G2", [P, NBT, R, P], F16, kind="ExternalInput").ap()
    x8_d = nc.dram_tensor("X8", [P, NBT, E, 2, 2, P], F8, kind="ExternalInput").ap()
    WP_d = nc.dram_tensor("WP16", [P, R, FC, D], F16, kind="ExternalInput").ap()
    W8_d = nc.dram_tensor("W8", [P, E, 2, 2, D], F8, kind="ExternalInput").ap()
    y_d = nc.dram_tensor("y", [B_LOC, D], F16, kind="ExternalOutput").ap()

    with tile.TileContext(nc) as tc:
        with (
            tc.tile_pool(name="const", bufs=1) as const_pool,
            tc.tile_pool(name="xtp", bufs=6) as xt_pool,
            tc.tile_pool(name="gp", bufs=6) as g_pool,
            tc.tile_pool(name="x8p", bufs=6) as x8_pool,
            tc.tile_pool(name="xsp", bufs=6) as xs_pool,
            tc.tile_pool(name="tdp", bufs=3) as td_pool,
            tc.tile_pool(name="yout", bufs=3) as y_pool,
            tc.tile_pool(name="zpsum", bufs=8, space="PSUM") as z_pool,
        ):
            junk_l = const_pool.tile([P, P], F16, name="junk_l")
            junk_r = const_pool.tile([P, D], F16, name="junk_r")
            nc.vector.memset(junk_l[:], 0.0)
            nc.vector.memset(junk_r[:], 0.0)

            # Pseudo-expert weights first (first matmuls need them), then
            # residual fp8 weights one transfer per expert.
            WP_sb = const_pool.tile([P, R, FC, D], F16, name="WP_sb")
            nc.scalar.dma_start(out=WP_sb[:, 0, 0], in_=WP_d[:, 0, 0])
            nc.scalar.dma_start(out=WP_sb[:, 0, 1:], in_=WP_d[:, 0, 1:])
            nc.scalar.dma_start(out=WP_sb[:, 1], in_=WP_d[:, 1])
            W8_sb = const_pool.tile([P, E, 2, 2, D], F8, name="W8_sb")
            for e in range(E):
                nc.scalar.dma_start(out=W8_sb[:, e], in_=W8_d[:, e])

            def load_tile(bt):
                xt = xt_pool.tile([P, FC, P], F16, name="xt", tag="xt")
                nc.sync.dma_start(out=xt[:], in_=xt_d[:, bt])
                gt = g_pool.tile([P, R, P], F16, name="gt", tag="gt")
                nc.sync.dma_start(out=gt[:], in_=g_d[:, bt])
                x8 = x8_pool.tile([P, E, 2, 2, P], F8, name="x8", tag="x8")
                nc.sync.dma_start(out=x8[:], in_=x8_d[:, bt])
                return xt, gt, x8

            def scale_tile(xt, gt, eng=None):
                # Xp[p, fc, r, b] = xt[p, fc, b] * gt[p, r, b]
                eng = eng or nc.vector
                xp = xs_pool.tile([P, FC, R, P], F16, name="xp", tag="xp")
                for fc in range(FC):
                    eng.tensor_mul(
                        out=xp[:, fc],
                        in0=xt[:, fc, None, :].to_broadcast([P, R, P]),
                        in1=gt[:],
                    )
                return xp

            def scale_tile_split(xt, gt):
                # Rank-split head muls: all r0 slices first so round r0's
                # dependency chain is half a mul per (tile, fc).
                xp = xs_pool.tile([P, FC, R, P], F16, name="xp", tag="xp")
                for r in range(R):
                    for fc in range(FC):
                        nc.vector.tensor_mul(
                            out=xp[:, fc, r : r + 1],
                            in0=xt[:, fc, None, :].to_broadcast([P, 1, P]),
                            in1=gt[:, r : r + 1],
                        )
                return xp

            def mm_main(ps_m, xp, r, first, last):
                for fc in range(FC):
                    nc.tensor.matmul(
                        ps_m[:],
                        lhsT=xp[:, fc, r, :],
                        rhs=WP_sb[:, r, fc, :],
                        start=(first and fc == 0),
                        stop=(last and fc == FC - 1),
                    )

            def mm_delta(ps_d, x8, e, first, last):
                for j in range(2):
                    nc.tensor.matmul(
                        ps_d[:],
                        lhsT=x8[:, e, j],
                        rhs=W8_sb[:, e, j],
                        start=(first and j == 0),
                        stop=(last and j == 1),
                        perf_mode=DR,
                    )

            def store_tile(bt, ps_m, ps_d):
                td = td_pool.tile([P, D], F16, name="td", tag="td")
                nc.scalar.mul(td[:], ps_d[:], EVAC)
                y_t = y_pool.tile([P, D], F16, name="y_t")
                nc.vector.tensor_add(out=y_t[:], in0=ps_m[:], in1=td[:])
                nc.scalar.dma_start(out=y_d[bt * P : (bt + 1) * P, :], in_=y_t[:])

            # --- Head: expert-outer rounds over HOIST tiles. Head x8 loads
            # split into expert halves so the first delta rounds (e0-3)
            # unblock after half the bytes.
            head = []
            for bt in range(HOIST):
                xt = xt_pool.tile([P, FC, P], F16, name="xt", tag="xt")
                nc.sync.dma_start(out=xt[:], in_=xt_d[:, bt])
                gt = g_pool.tile([P, R, P], F16, name="gt", tag="gt")
                nc.sync.dma_start(out=gt[:], in_=g_d[:, bt])
                x8 = x8_pool.tile([P, E, 2, 2, P], F8, name="x8", tag="x8")
                xp = scale_tile_split(xt, gt)
                ps_m = z_pool.tile([P, D], F32, name="psm", tag="ps")
                ps_d = z_pool.tile([P, D], F32, name="psd", tag="ps")
                head.append((xp, x8, ps_m, ps_d))
            for q in range(4):
                for bt in range(HOIST):
                    nc.sync.dma_start(
                        out=head[bt][1][:, 2 * q : 2 * q + 2],
                        in_=x8_d[:, bt, 2 * q : 2 * q + 2],
                    )

            # Prewarm: zeros accumulated into tile 0's main bank (exact
            # no-op); tile 0's real chain continues with start=False.
            for i in range(NWARM):
                nc.tensor.matmul(
                    head[0][2][:], lhsT=junk_l[:], rhs=junk_r[:],
                    start=(i == 0), stop=False,
                )

            for r in range(R):
                for bt in range(HOIST):
                    mm_main(head[bt][2], head[bt][0], r,
                            first=(r == 0 and bt != 0), last=(r == R - 1))
            for e in range(E):
                for bt in range(HOIST):
                    mm_delta(head[bt][3], head[bt][1], e,
                             first=(e == 0), last=(e == E - 1))
            for bt in range(HOIST):
                store_tile(bt, head[bt][2], head[bt][3])

            # --- Steady state.
            for bt in range(HOIST, NBT - 1):
                xt, gt, x8 = load_tile(bt)
                xp = scale_tile(xt, gt)
                ps_m = z_pool.tile([P, D], F32, name="psm", tag="ps")
                ps_d = z_pool.tile([P, D], F32, name="psd", tag="ps")
                for r in range(R):
                    mm_main(ps_m, xp, r, first=(r == 0), last=(r == R - 1))
                for e in range(E):
                    mm_delta(ps_d, x8, e, first=(e == 0), last=(e == E - 1))
                store_tile(bt, ps_m, ps_d)

            # --- Last tile: two 256-wide output halves; the first half's
            # evacuation (ACT scale-copy + DVE add + store) overlaps the
            # second half's matmuls, shortening the kernel tail.
            bt = NBT - 1
            xt, gt, x8 = load_tile(bt)
            xp = scale_tile(xt, gt)
            y_t = y_pool.tile([P, D], F16, name="y_t")
            for h in range(2):
                lo, hi = h * 256, (h + 1) * 256
                pm = z_pool.tile([P, D // 2], F32, name="pmh", tag="ps")
                pd = z_pool.tile([P, D // 2], F32, name="pdh", tag="ps")
                for r in range(R):
                    for fc in range(FC):
                        nc.tensor.matmul(
                            pm[:], lhsT=xp[:, fc, r, :],
                            rhs=WP_sb[:, r, fc, lo:hi],
                            start=(r == 0 and fc == 0),
                            stop=(r == R - 1 and fc == FC - 1),
                        )
                for e in range(E):
                    for j in range(2):
                        nc.tensor.matmul(
                            pd[:], lhsT=x8[:, e, j],
                            rhs=W8_sb[:, e, j, :, lo:hi],
                            start=(e == 0 and j == 0),
                            stop=(e == E - 1 and j == 1),
                            perf_mode=DR,
                        )
                td = td_pool.tile([P, D // 2], F16, name="tdh", tag="tdh")
                nc.scalar.mul(td[:], pd[:], EVAC)
                nc.vector.tensor_add(out=y_t[:, lo:hi], in0=pm[:], in1=td[:])
                nc.sync.dma_start(
                    out=y_d[bt * P : (bt + 1) * P, lo:hi], in_=y_t[:, lo:hi]
                )

    nc.compile()
    return nc


def _get_nc():
    if "nc" not in _COMPILED:
        _COMPILED["nc"] = _build_nc()
    return _COMPILED["nc"]


def prep_inputs(x, weights, W):
    """Host-side shard + preprocess: returns per-core input maps."""
    import ml_dtypes

    x = np.asarray(x, dtype=np.float32)
    weights = np.asarray(weights, dtype=np.float32)
    W = np.asarray(W, dtype=np.float32)

    U, S, Vt = np.linalg.svd(weights, full_matrices=False)
    G = U[:, :R] * S[:R]                      # [B, R] pseudo-gates
    res = weights - G @ Vt[:R]                # [B, E] residual gates
    WP = np.einsum("re,eio->rio", Vt[:R], W)  # [R, 512, 512]

    # WP16[p, r, fc, o] = WP[r, fc*128+p, o]
    WP16 = np.ascontiguousarray(
        WP.reshape(R, FC, P, D).transpose(2, 0, 1, 3).astype(np.float16)
    )
    # W8[p, e, j, ko, o] = W[e, j*256+ko*128+p, o] * 2^15
    W8 = np.ascontiguousarray(
        np.clip(W.reshape(E, 2, 2, P, D).transpose(3, 0, 1, 2, 4) * SW,
                -240.0, 240.0).astype(ml_dtypes.float8_e4m3)
    )

    xs = x.reshape(N_CORES, NBT, P, FC, P)
    xs_flat = x.reshape(N_CORES, B_LOC, D)
    gs = G.reshape(N_CORES, NBT, P, R)
    rs = res.reshape(N_CORES, B_LOC, E)
    in_maps = []
    for c in range(N_CORES):
        xt = np.ascontiguousarray(
            xs[c].transpose(3, 0, 2, 1).astype(np.float16)
        )
        g2 = np.ascontiguousarray(
            np.broadcast_to(
                gs[c].transpose(0, 2, 1)[None], (P, NBT, R, P)
            ).astype(np.float16)
        )
        # X8[p, t, e, j, ko, b] = x[t*128+b, j*256+ko*128+p]*res[t*128+b, e]*32
        t8 = (
            xs_flat[c][:, None, :] * rs[c][:, :, None] * SX
        )  # [B_LOC, E, D]
        t8 = np.clip(t8, -240.0, 240.0).astype(ml_dtypes.float8_e4m3)
        t8 = t8.reshape(NBT, P, E, 2, 2, P)          # [t, b, e, j, ko, p]
        x8 = np.ascontiguousarray(t8.transpose(5, 0, 2, 3, 4, 1))
        in_maps.append(
            {"XT": xt, "G2": g2, "X8": x8, "WP16": WP16, "W8": W8}
        )
    return in_maps


def kernel(x, weights, W, b):
    from concourse.bass_utils import run_bass_kernel_spmd

    b_np = np.asarray(b, dtype=np.float32)
    nc = _get_nc()
    in_maps = prep_inputs(x, weights, W)
    res = run_bass_kernel_spmd(nc, in_maps, core_ids=list(range(N_CORES)))
    y = np.concatenate(
        [res.results[c]["y"].astype(np.float32) for c in range(N_CORES)], axis=0
    )

    if np.any(b_np):
        y = y + np.asarray(weights, dtype=np.float32) @ b_np[:, 0, :]

    return y.astype(np.float32)


# revision 37
# speedup vs baseline: 1.0039x; 1.0039x over previous
"""Trainium2 Bass kernel for nn_ExpertsLinear (weighted mixture of 8 experts).

    y[b, o] = sum_e weights[b, e] * (x @ W[e] + b[e])[b, o]

Split-precision formulation. The gate matrix w [B, 8] is split host-side
via rank-2 SVD: w = G @ V + res (G = U[:, :2]*S[:2], V = Vt[:2]).

    y_b = sum_r G_br * (x_b @ W'_r)            # fp16, W'_r = sum_e V_re W_e
        + sum_e res_be * (x_b @ W_e)           # fp8-e4m3 DoubleRow, 2x rate

The fp16 term carries ~87% of the signal; the fp8 residual term's
quantization error lands at l2_rel ~1.7e-2 (gate 2e-2, simulated with
exact kernel quantization). DoubleRow packs 2 fp8 k-values per PE cell:
lhsT [K,2,M], rhs [K,2,N], contraction over (k, pair) — verified on HW.

Host-side preprocessing: SVD of w; x pre-transposed fp16; residual-gated
x pre-scaled (*32, clip +-240) and packed fp8; W packed fp8 (*2^15);
pseudo-expert weights/gates fp16. Scales divided out at PSUM evacuation
(ACT scale-copy + DVE add), y stored fp16.

Per-core, per 128-row tile: 8 fp16 MMs (2 pseudo-experts) into one PSUM
bank + 16 DoubleRow fp8 MMs (8 residual experts) into a second bank,
then y = ps_main + ps_delta * 2^-20. Head: expert-outer rounds over
HOIST tiles while weights stream; zero-matmul prewarm bridges the
initial all-cores HBM burst and warms the HAM clock gate.
"""

import numpy as np

P = 128
D = 512
E = 8
R = 2
FC = D // P
N_CORES = 8
B_FULL = 65536
B_LOC = B_FULL // N_CORES
NBT = B_LOC // P

HOIST = 4
NWARM = 13
SX = 32.0
SW = 2.0 ** 15
EVAC = 1.0 / (SX * SW)

_COMPILED = {}


def _build_nc():
    import concourse.bacc as bacc
    import concourse.mybir as mybir
    import concourse.tile as tile

    F32 = mybir.dt.float32
    F16 = mybir.dt.float16
    F8 = mybir.dt.float8e4
    DR = mybir.MatmulPerfMode.DoubleRow

    nc = bacc.Bacc(
        "TRN2",
        target_bir_lowering=False,
        debug=False,
        enable_asserts=False,
        num_devices=N_CORES,
    )
    xt_d = nc.dram_tensor("XT", [P, NBT, FC, P], F16, kind="ExternalInput").ap()
    g_d = nc.dram_tensor("G2", [P, NBT, R, P], F16, kind="ExternalInput").ap()
    x8_d = nc.dram_tensor("X8", [P, NBT, E, 2, 2, P], F8, kind="ExternalInput").ap()
    WP_d = nc.dram_tensor("WP16", [P, R, FC, D], F16, kind="ExternalInput").ap()
    W8_d = nc.dram_tensor("W8", [P, E, 2, 2, D], F8, kind="ExternalInput").ap()
    y_d = nc.dram_tensor("y", [B_LOC, D], F16, kind="ExternalOutput").ap()

    with tile.TileContext(nc) as tc:
        with (
            tc.tile_pool(name="const", bufs=1) as const_pool,
            tc.tile_pool(name="xtp", bufs=6) as xt_pool,
            tc.tile_pool(name="gp", bufs=6) as g_pool,
            tc.tile_pool(name="x8p", bufs=6) as x8_pool,
            tc.tile_pool(name="xsp", bufs=6) as xs_pool,
            tc.tile_pool(name="tdp", bufs=3) as td_pool,
            tc.tile_pool(name="yout", bufs=3) as y_pool,
            tc.tile_pool(name="zpsum", bufs=8, space="PSUM") as z_pool,
        ):
            junk_l = const_pool.tile([P, P], F16, name="junk_l")
            junk_r = const_pool.tile([P, D], F16, name="junk_r")
            nc.vector.memset(junk_l[:], 0.0)
            nc.vector.memset(junk_r[:], 0.0)

            # Pseudo-expert weights first (first matmuls need them), then
            # residual fp8 weights one transfer per expert.
            WP_sb = const_pool.tile([P, R, FC, D], F16, name="WP_sb")
            nc.scalar.dma_start(out=WP_sb[:, 0, 0], in_=WP_d[:, 0, 0])
            nc.scalar.dma_start(out=WP_sb[:, 0, 1:], in_=WP_d[:, 0, 1:])
            nc.scalar.dma_start(out=WP_sb[:, 1], in_=WP_d[:, 1])
            W8_sb = const_pool.tile([P, E, 2, 2, D], F8, name="W8_sb")
            for e in range(E):
                nc.scalar.dma_start(out=W8_sb[:, e], in_=W8_d[:, e])

            def load_tile(bt):
                xt = xt_pool.tile([P, FC, P], F16, name="xt", tag="xt")
                nc.sync.dma_start(out=xt[:], in_=xt_d[:, bt])
                gt = g_pool.tile([P, R, P], F16, name="gt", tag="gt")
                nc.sync.dma_start(out=gt[:], in_=g_d[:, bt])
                x8 = x8_pool.tile([P, E, 2, 2, P], F8, name="x8", tag="x8")
                nc.sync.dma_start(out=x8[:], in_=x8_d[:, bt])
                return xt, gt, x8

            def scale_tile(xt, gt, eng=None):
                # Xp[p, fc, r, b] = xt[p, fc, b] * gt[p, r, b]
                eng = eng or nc.vector
                xp = xs_pool.tile([P, FC, R, P], F16, name="xp", tag="xp")
                for fc in range(FC):
                    eng.tensor_mul(
                        out=xp[:, fc],
                        in0=xt[:, fc, None, :].to_broadcast([P, R, P]),
                        in1=gt[:],
                    )
                return xp

            def scale_rank(xt, gt, xp, r):
                # One rank's gate muls; head emits r0 for ALL tiles before
                # any r1 so round r0 never waits behind r1 muls on DVE.
                for fc in range(FC):
                    nc.vector.tensor_mul(
                        out=xp[:, fc, r : r + 1],
                        in0=xt[:, fc, None, :].to_broadcast([P, 1, P]),
                        in1=gt[:, r : r + 1],
                    )

            def mm_main(ps_m, xp, r, first, last):
                for fc in range(FC):
                    nc.tensor.matmul(
                        ps_m[:],
                        lhsT=xp[:, fc, r, :],
                        rhs=WP_sb[:, r, fc, :],
                        start=(first and fc == 0),
                        stop=(last and fc == FC - 1),
                    )

            def mm_delta(ps_d, x8, e, first, last):
                for j in range(2):
                    nc.tensor.matmul(
                        ps_d[:],
                        lhsT=x8[:, e, j],
                        rhs=W8_sb[:, e, j],
                        start=(first and j == 0),
                        stop=(last and j == 1),
                        perf_mode=DR,
                    )

            def store_tile(bt, ps_m, ps_d):
                td = td_pool.tile([P, D], F16, name="td", tag="td")
                nc.scalar.mul(td[:], ps_d[:], EVAC)
                y_t = y_pool.tile([P, D], F16, name="y_t")
                nc.vector.tensor_add(out=y_t[:], in0=ps_m[:], in1=td[:])
                nc.scalar.dma_start(out=y_d[bt * P : (bt + 1) * P, :], in_=y_t[:])

            # --- Head: expert-outer rounds over HOIST tiles. Head x8 loads
            # split into expert halves so the first delta rounds (e0-3)
            # unblock after half the bytes.
            head = []
            for bt in range(HOIST):
                xt = xt_pool.tile([P, FC, P], F16, name="xt", tag="xt")
                nc.sync.dma_start(out=xt[:], in_=xt_d[:, bt])
                gt = g_pool.tile([P, R, P], F16, name="gt", tag="gt")
                nc.sync.dma_start(out=gt[:], in_=g_d[:, bt])
                x8 = x8_pool.tile([P, E, 2, 2, P], F8, name="x8", tag="x8")
                xp = xs_pool.tile([P, FC, R, P], F16, name="xp", tag="xp")
                scale_rank(xt, gt, xp, 0)
                ps_m = z_pool.tile([P, D], F32, name="psm", tag="ps")
                ps_d = z_pool.tile([P, D], F32, name="psd", tag="ps")
                head.append((xp, x8, ps_m, ps_d, xt, gt))
            for bt in range(HOIST):
                scale_rank(head[bt][4], head[bt][5], head[bt][0], 1)
            for q in range(4):
                for bt in range(HOIST):
                    nc.sync.dma_start(
                        out=head[bt][1][:, 2 * q : 2 * q + 2],
                        in_=x8_d[:, bt, 2 * q : 2 * q + 2],
                    )

            # Prewarm: zeros accumulated into tile 0's main bank (exact
            # no-op); tile 0's real chain continues with start=False.
            for i in range(NWARM):
                nc.tensor.matmul(
                    head[0][2][:], lhsT=junk_l[:], rhs=junk_r[:],
                    start=(i == 0), stop=False,
                )

            for r in range(R):
                for bt in range(HOIST):
                    mm_main(head[bt][2], head[bt][0], r,
                            first=(r == 0 and bt != 0), last=(r == R - 1))
            for e in range(E):
                for bt in range(HOIST):
                    mm_delta(head[bt][3], head[bt][1], e,
                             first=(e == 0), last=(e == E - 1))
            for bt in range(HOIST):
                store_tile(bt, head[bt][2], head[bt][3])

            # --- Steady state.
            for bt in range(HOIST, NBT - 1):
                xt, gt, x8 = load_tile(bt)
                xp = scale_tile(xt, gt)
                ps_m = z_pool.tile([P, D], F32, name="psm", tag="ps")
                ps_d = z_pool.tile([P, D], F32, name="psd", tag="ps")
                for r in range(R):
                    mm_main(ps_m, xp, r, first=(r == 0), last=(r == R - 1))
                for e in range(E):
                    mm_delta(ps_d, x8, e, first=(e == 0), last=(e == E - 1))
                store_tile(bt, ps_m, ps_d)

            # --- Last tile: two 256-wide output halves; the first half's
            # evacuation (ACT scale-copy + DVE add + store) overlaps the
            # second half's matmuls, shortening the kernel tail.
            bt = NBT - 1
            xt, gt, x8 = load_tile(bt)
            xp = scale_tile(xt, gt)
            y_t = y_pool.tile([P, D], F16, name="y_t")
            for h in range(2):
                lo, hi = h * 256, (h + 1) * 256
                pm = z_pool.tile([P, D // 2], F32, name="pmh", tag="ps")
                pd = z_pool.tile([P, D // 2], F32, name="pdh", tag="ps")
                for r in range(R):
                    for fc in range(FC):
                        nc.tensor.matmul(
                            pm[:], lhsT=xp[:, fc, r, :],
                            rhs=WP_sb[:, r, fc, lo:hi],
                            start=(r == 0 and fc == 0),
                            stop=(r == R - 1 and fc == FC - 1),
                        )
                for e in range(E):
                    for j in range(2):
                        nc.tensor.matmul(
                            pd[:], lhsT=x8[:, e, j],
                            rhs=W8_sb[:, e, j, :, lo:hi],
                            start=(e == 0 and j == 0),
                            stop=(e == E - 1 and j == 1),
                            perf_mode=DR,
                        )
                td = td_pool.tile([P, D // 2], F16, name="tdh", tag="tdh")
                nc.scalar.mul(td[:], pd[:], EVAC)
                nc.vector.tensor_add(out=y_t[:, lo:hi], in0=pm[:], in1=td[:])
                nc.sync.dma_start(
                    out=y_d[bt * P : (bt + 1) * P, lo:hi], in_=y_t[:, lo:hi]
                )

    nc.compile()
    return nc


def _get_nc():
    if "nc" not in _COMPILED:
        _COMPILED["nc"] = _build_nc()
    return _COMPILED["nc"]


def prep_inputs(x, weights, W):
    """Host-side shard + preprocess: returns per-core input maps."""
    import ml_dtypes

    x = np.asarray(x, dtype=np.float32)
    weights = np.asarray(weights, dtype=np.float32)
    W = np.asarray(W, dtype=np.float32)

    U, S, Vt = np.linalg.svd(weights, full_matrices=False)
    G = U[:, :R] * S[:R]                      # [B, R] pseudo-gates
    res = weights - G @ Vt[:R]                # [B, E] residual gates
    WP = np.einsum("re,eio->rio", Vt[:R], W)  # [R, 512, 512]

    # WP16[p, r, fc, o] = WP[r, fc*128+p, o]
    WP16 = np.ascontiguousarray(
        WP.reshape(R, FC, P, D).transpose(2, 0, 1, 3).astype(np.float16)
    )
    # W8[p, e, j, ko, o] = W[e, j*256+ko*128+p, o] * 2^15
    W8 = np.ascontiguousarray(
        np.clip(W.reshape(E, 2, 2, P, D).transpose(3, 0, 1, 2, 4) * SW,
                -240.0, 240.0).astype(ml_dtypes.float8_e4m3)
    )

    xs = x.reshape(N_CORES, NBT, P, FC, P)
    xs_flat = x.reshape(N_CORES, B_LOC, D)
    gs = G.reshape(N_CORES, NBT, P, R)
    rs = res.reshape(N_CORES, B_LOC, E)
    in_maps = []
    for c in range(N_CORES):
        xt = np.ascontiguousarray(
            xs[c].transpose(3, 0, 2, 1).astype(np.float16)
        )
        g2 = np.ascontiguousarray(
            np.broadcast_to(
                gs[c].transpose(0, 2, 1)[None], (P, NBT, R, P)
            ).astype(np.float16)
        )
        # X8[p, t, e, j, ko, b] = x[t*128+b, j*256+ko*128+p]*res[t*128+b, e]*32
        t8 = (
            xs_flat[c][:, None, :] * rs[c][:, :, None] * SX
        )  # [B_LOC, E, D]
        t8 = np.clip(t8, -240.0, 240.0).astype(ml_dtypes.float8_e4m3)
        t8 = t8.reshape(NBT, P, E, 2, 2, P)          # [t, b, e, j, ko, p]
        x8 = np.ascontiguousarray(t8.transpose(5, 0, 2, 3, 4, 1))
        in_maps.append(
            {"XT": xt, "G2": g2, "X8": x8, "WP16": WP16, "W8": W8}
        )
    return in_maps


def kernel(x, weights, W, b):
    from concourse.bass_utils import run_bass_kernel_spmd

    b_np = np.asarray(b, dtype=np.float32)
    nc = _get_nc()
    in_maps = prep_inputs(x, weights, W)
    res = run_bass_kernel_spmd(nc, in_maps, core_ids=list(range(N_CORES)))
    y = np.concatenate(
        [res.results[c]["y"].astype(np.float32) for c in range(N_CORES)], axis=0
    )

    if np.any(b_np):
        y = y + np.asarray(weights, dtype=np.float32) @ b_np[:, 0, :]

    return y.astype(np.float32)


# revision 38
# speedup vs baseline: 1.0071x; 1.0032x over previous
"""Trainium2 Bass kernel for nn_ExpertsLinear (weighted mixture of 8 experts).

    y[b, o] = sum_e weights[b, e] * (x @ W[e] + b[e])[b, o]

Split-precision formulation. The gate matrix w [B, 8] is split host-side
via rank-2 SVD: w = G @ V + res (G = U[:, :2]*S[:2], V = Vt[:2]).

    y_b = sum_r G_br * (x_b @ W'_r)            # fp16, W'_r = sum_e V_re W_e
        + sum_e res_be * (x_b @ W_e)           # fp8-e4m3 DoubleRow, 2x rate

The fp16 term carries ~87% of the signal; the fp8 residual term's
quantization error lands at l2_rel ~1.7e-2 (gate 2e-2, simulated with
exact kernel quantization). DoubleRow packs 2 fp8 k-values per PE cell:
lhsT [K,2,M], rhs [K,2,N], contraction over (k, pair) — verified on HW.

Host-side preprocessing: SVD of w; x pre-transposed fp16; residual-gated
x pre-scaled (*32, clip +-240) and packed fp8; W packed fp8 (*2^15);
pseudo-expert weights/gates fp16. Scales divided out at PSUM evacuation
(ACT scale-copy + DVE add), y stored fp16.

Per-core, per 128-row tile: 8 fp16 MMs (2 pseudo-experts) into one PSUM
bank + 16 DoubleRow fp8 MMs (8 residual experts) into a second bank,
then y = ps_main + ps_delta * 2^-20. Head: expert-outer rounds over
HOIST tiles while weights stream; zero-matmul prewarm bridges the
initial all-cores HBM burst and warms the HAM clock gate.
"""

import numpy as np

P = 128
D = 512
E = 8
R = 2
FC = D // P
N_CORES = 8
B_FULL = 65536
B_LOC = B_FULL // N_CORES
NBT = B_LOC // P

HOIST = 4
NWARM = 13
SX = 32.0
SW = 2.0 ** 15
EVAC = 1.0 / (SX * SW)

_COMPILED = {}


def _build_nc():
    import concourse.bacc as bacc
    import concourse.mybir as mybir
    import concourse.tile as tile

    F32 = mybir.dt.float32
    F16 = mybir.dt.float16
    F8 = mybir.dt.float8e4
    DR = mybir.MatmulPerfMode.DoubleRow

    nc = bacc.Bacc(
        "TRN2",
        target_bir_lowering=False,
        debug=False,
        enable_asserts=False,
        num_devices=N_CORES,
    )
    xt_d = nc.dram_tensor("XT", [P, NBT, FC, P], F16, kind="ExternalInput").ap()
    g_d = nc.dram_tensor("G2", [P, NBT, R, P], F16, kind="ExternalInput").ap()
    x8_d = nc.dram_tensor("X8", [P, NBT, E, 2, 2, P], F8, kind="ExternalInput").ap()
    WP_d = nc.dram_tensor("WP16", [P, R, FC, D], F16, kind="ExternalInput").ap()
    W8_d = nc.dram_tensor("W8", [P, E, 2, 2, D], F8, kind="ExternalInput").ap()
    y_d = nc.dram_tensor("y", [B_LOC, D], F16, kind="ExternalOutput").ap()

    with tile.TileContext(nc) as tc:
        with (
            tc.tile_pool(name="const", bufs=1) as const_pool,
            tc.tile_pool(name="xtp", bufs=6) as xt_pool,
            tc.tile_pool(name="gp", bufs=6) as g_pool,
            tc.tile_pool(name="x8p", bufs=6) as x8_pool,
            tc.tile_pool(name="xsp", bufs=6) as xs_pool,
            tc.tile_pool(name="tdp", bufs=3) as td_pool,
            tc.tile_pool(name="yout", bufs=3) as y_pool,
            tc.tile_pool(name="zpsum", bufs=8, space="PSUM") as z_pool,
        ):
            junk_l = const_pool.tile([P, P], F16, name="junk_l")
            junk_r = const_pool.tile([P, D], F16, name="junk_r")
            nc.vector.memset(junk_l[:], 0.0)
            nc.vector.memset(junk_r[:], 0.0)

            # Pseudo-expert weights first (first matmuls need them), then
            # residual fp8 weights one transfer per expert.
            WP_sb = const_pool.tile([P, R, FC, D], F16, name="WP_sb")
            nc.scalar.dma_start(out=WP_sb[:, 0, 0], in_=WP_d[:, 0, 0])
            nc.scalar.dma_start(out=WP_sb[:, 0, 1:], in_=WP_d[:, 0, 1:])
            nc.scalar.dma_start(out=WP_sb[:, 1], in_=WP_d[:, 1])
            W8_sb = const_pool.tile([P, E, 2, 2, D], F8, name="W8_sb")
            for e in range(E):
                nc.scalar.dma_start(out=W8_sb[:, e], in_=W8_d[:, e])

            def load_tile(bt):
                xt = xt_pool.tile([P, FC, P], F16, name="xt", tag="xt")
                nc.sync.dma_start(out=xt[:], in_=xt_d[:, bt])
                gt = g_pool.tile([P, R, P], F16, name="gt", tag="gt")
                nc.sync.dma_start(out=gt[:], in_=g_d[:, bt])
                x8 = x8_pool.tile([P, E, 2, 2, P], F8, name="x8", tag="x8")
                nc.sync.dma_start(out=x8[:], in_=x8_d[:, bt])
                return xt, gt, x8

            def scale_tile(xt, gt, eng=None):
                # Xp[p, fc, r, b] = xt[p, fc, b] * gt[p, r, b]
                eng = eng or nc.vector
                xp = xs_pool.tile([P, FC, R, P], F16, name="xp", tag="xp")
                for fc in range(FC):
                    eng.tensor_mul(
                        out=xp[:, fc],
                        in0=xt[:, fc, None, :].to_broadcast([P, R, P]),
                        in1=gt[:],
                    )
                return xp

            def scale_rank(xt, gt, xp, r):
                # One rank's gate muls; head emits r0 for ALL tiles before
                # any r1 so round r0 never waits behind r1 muls on DVE.
                for fc in range(FC):
                    nc.vector.tensor_mul(
                        out=xp[:, fc, r : r + 1],
                        in0=xt[:, fc, None, :].to_broadcast([P, 1, P]),
                        in1=gt[:, r : r + 1],
                    )

            def mm_main(ps_m, xp, r, first, last):
                for fc in range(FC):
                    nc.tensor.matmul(
                        ps_m[:],
                        lhsT=xp[:, fc, r, :],
                        rhs=WP_sb[:, r, fc, :],
                        start=(first and fc == 0),
                        stop=(last and fc == FC - 1),
                    )

            def mm_delta(ps_d, x8, e, first, last):
                for j in range(2):
                    nc.tensor.matmul(
                        ps_d[:],
                        lhsT=x8[:, e, j],
                        rhs=W8_sb[:, e, j],
                        start=(first and j == 0),
                        stop=(last and j == 1),
                        perf_mode=DR,
                    )

            def store_tile(bt, ps):
                y_t = y_pool.tile([P, D], F16, name="y_t")
                nc.scalar.mul(y_t[:], ps[:], EVAC)
                nc.scalar.dma_start(out=y_d[bt * P : (bt + 1) * P, :], in_=y_t[:])

            # --- Head: expert-outer rounds over HOIST tiles. Head x8 loads
            # split into expert halves so the first delta rounds (e0-3)
            # unblock after half the bytes.
            head = []
            for bt in range(HOIST):
                xt = xt_pool.tile([P, FC, P], F16, name="xt", tag="xt")
                nc.sync.dma_start(out=xt[:], in_=xt_d[:, bt])
                gt = g_pool.tile([P, R, P], F16, name="gt", tag="gt")
                nc.sync.dma_start(out=gt[:], in_=g_d[:, bt])
                x8 = x8_pool.tile([P, E, 2, 2, P], F8, name="x8", tag="x8")
                xp = xs_pool.tile([P, FC, R, P], F16, name="xp", tag="xp")
                scale_rank(xt, gt, xp, 0)
                ps = z_pool.tile([P, D], F32, name="ps", tag="ps")
                head.append((xp, x8, ps, xt, gt))
            for bt in range(HOIST):
                scale_rank(head[bt][3], head[bt][4], head[bt][0], 1)
            for q in range(4):
                for bt in range(HOIST):
                    nc.sync.dma_start(
                        out=head[bt][1][:, 2 * q : 2 * q + 2],
                        in_=x8_d[:, bt, 2 * q : 2 * q + 2],
                    )

            # Prewarm: zeros accumulated into tile 0's main bank (exact
            # no-op); tile 0's real chain continues with start=False.
            for i in range(NWARM):
                nc.tensor.matmul(
                    head[0][2][:], lhsT=junk_l[:], rhs=junk_r[:],
                    start=(i == 0), stop=False,
                )

            for r in range(R):
                for bt in range(HOIST):
                    mm_main(head[bt][2], head[bt][0], r,
                            first=(r == 0 and bt != 0), last=False)
            for e in range(E):
                for bt in range(HOIST):
                    mm_delta(head[bt][2], head[bt][1], e,
                             first=False, last=(e == E - 1))
            for bt in range(HOIST):
                store_tile(bt, head[bt][2])

            # --- Steady state.
            for bt in range(HOIST, NBT - 1):
                xt, gt, x8 = load_tile(bt)
                xp = scale_tile(xt, gt)
                ps = z_pool.tile([P, D], F32, name="ps", tag="ps")
                for r in range(R):
                    mm_main(ps, xp, r, first=(r == 0), last=False)
                for e in range(E):
                    mm_delta(ps, x8, e, first=False, last=(e == E - 1))
                store_tile(bt, ps)

            # --- Last tile: two 256-wide output halves; the first half's
            # evacuation (ACT scale-copy + DVE add + store) overlaps the
            # second half's matmuls, shortening the kernel tail.
            bt = NBT - 1
            xt, gt, x8 = load_tile(bt)
            xp = scale_tile(xt, gt)
            y_t = y_pool.tile([P, D], F16, name="y_t")
            for h in range(2):
                lo, hi = h * 256, (h + 1) * 256
                ph = z_pool.tile([P, D // 2], F32, name="ph", tag="ps")
                for r in range(R):
                    for fc in range(FC):
                        nc.tensor.matmul(
                            ph[:], lhsT=xp[:, fc, r, :],
                            rhs=WP_sb[:, r, fc, lo:hi],
                            start=(r == 0 and fc == 0),
                            stop=False,
                        )
                for e in range(E):
                    for j in range(2):
                        nc.tensor.matmul(
                            ph[:], lhsT=x8[:, e, j],
                            rhs=W8_sb[:, e, j, :, lo:hi],
                            start=False,
                            stop=(e == E - 1 and j == 1),
                            perf_mode=DR,
                        )
                nc.vector.tensor_scalar_mul(y_t[:, lo:hi], ph[:], EVAC)
                nc.sync.dma_start(
                    out=y_d[bt * P : (bt + 1) * P, lo:hi], in_=y_t[:, lo:hi]
                )

    nc.compile()
    return nc


def _get_nc():
    if "nc" not in _COMPILED:
        _COMPILED["nc"] = _build_nc()
    return _COMPILED["nc"]


def prep_inputs(x, weights, W):
    """Host-side shard + preprocess: returns per-core input maps."""
    import ml_dtypes

    x = np.asarray(x, dtype=np.float32)
    weights = np.asarray(weights, dtype=np.float32)
    W = np.asarray(W, dtype=np.float32)

    U, S, Vt = np.linalg.svd(weights, full_matrices=False)
    G = U[:, :R] * S[:R]                      # [B, R] pseudo-gates
    Gs = G * 256.0                            # 2^8: main term scaled 2^20 like delta
    res = weights - G @ Vt[:R]                # [B, E] residual gates
    WP = np.einsum("re,eio->rio", Vt[:R], W) * 4096.0  # [R,512,512] * 2^12

    # WP16[p, r, fc, o] = WP[r, fc*128+p, o]
    WP16 = np.ascontiguousarray(
        WP.reshape(R, FC, P, D).transpose(2, 0, 1, 3).astype(np.float16)
    )
    # W8[p, e, j, ko, o] = W[e, j*256+ko*128+p, o] * 2^15
    W8 = np.ascontiguousarray(
        np.clip(W.reshape(E, 2, 2, P, D).transpose(3, 0, 1, 2, 4) * SW,
                -240.0, 240.0).astype(ml_dtypes.float8_e4m3)
    )

    xs = x.reshape(N_CORES, NBT, P, FC, P)
    xs_flat = x.reshape(N_CORES, B_LOC, D)
    gs = Gs.reshape(N_CORES, NBT, P, R)
    rs = res.reshape(N_CORES, B_LOC, E)
    in_maps = []
    for c in range(N_CORES):
        xt = np.ascontiguousarray(
            xs[c].transpose(3, 0, 2, 1).astype(np.float16)
        )
        g2 = np.ascontiguousarray(
            np.broadcast_to(
                gs[c].transpose(0, 2, 1)[None], (P, NBT, R, P)
            ).astype(np.float16)
        )
        # X8[p, t, e, j, ko, b] = x[t*128+b, j*256+ko*128+p]*res[t*128+b, e]*32
        t8 = (
            xs_flat[c][:, None, :] * rs[c][:, :, None] * SX
        )  # [B_LOC, E, D]
        t8 = np.clip(t8, -240.0, 240.0).astype(ml_dtypes.float8_e4m3)
        t8 = t8.reshape(NBT, P, E, 2, 2, P)          # [t, b, e, j, ko, p]
        x8 = np.ascontiguousarray(t8.transpose(5, 0, 2, 3, 4, 1))
        in_maps.append(
            {"XT": xt, "G2": g2, "X8": x8, "WP16": WP16, "W8": W8}
        )
    return in_maps


def kernel(x, weights, W, b):
    from concourse.bass_utils import run_bass_kernel_spmd

    b_np = np.asarray(b, dtype=np.float32)
    nc = _get_nc()
    in_maps = prep_inputs(x, weights, W)
    res = run_bass_kernel_spmd(nc, in_maps, core_ids=list(range(N_CORES)))
    y = np.concatenate(
        [res.results[c]["y"].astype(np.float32) for c in range(N_CORES)], axis=0
    )

    if np.any(b_np):
        y = y + np.asarray(weights, dtype=np.float32) @ b_np[:, 0, :]

    return y.astype(np.float32)
